# revision 1
# baseline (speedup 1.0000x reference)
"""DeepSeek-style MoE block (SwiGLU experts, top-k routing) on 8 Trainium2 cores.

v3 = the proven fp32r expert-parallel kernel structure, in bf16.

Expert-parallel sharding: each of the 8 cores owns E/8 = 2 experts and receives
only the tokens routed to those experts (host-side dispatch). Per expert e with
gathered/padded tokens XT [D, TG] (transposed):

    GT = W0e @ X^T          (PSUM f32, bf16 matmuls, DFF on partitions)
    UT = W1e @ X^T
    HT = silu(s0*GT) * UT   (SBUF bf16, [DFF, TG])
    Y  = (HT)^T @ W2e^T     (tokens on partitions)
    Yout = coef[token] * Y  where coef = s1*s2*cw  (cw = summed routing weights)

The host scatter-adds each expert's Yout rows into the dense [T, D] output.

Why bf16 (vs the fp32r baseline at 216 us): DMA bytes halve (63 -> ~31 MB per
core, under the 16-engine ~360 GB/s roofline that co-bottlenecked the fp32r
version), and bf16 matmuls stream at full 2.4 GHz with the implicit LDWEIGHTS
completely hidden (163 ns per 384-wide matmul) where fp32r paid a partial
stationary-reload penalty. PE busy is ~124 us/core = the GEMM floor.
rel_fro error ~4.4e-3 (bf16 rounding; verified vs numpy emulation) << 2e-2.

Schedule notes (each worth measured us on NTFF traces):
 - Bulk DMAs alternate nc.sync (HWDGE) + nc.gpsimd (SWDGE) so a launch that
   back-pressures on a full queue can't head-of-line-stall the Act/DVE
   sequencers, whose eltwise drains gate PSUM reuse and phase-2 start.
 - y outputs go via nc.sync only: a SWDGE dma in flight at program end costs
   ~10 us of gpsimd teardown drain.
 - Per-k xt loads interleave with the first f-group's weight blocks so the
   first matmul starts ~10 us in instead of waiting behind MBs of prefetch;
   w2t is partition-major (8 KB lines) and rides the ring FIFO so it doesn't
   steal prologue bandwidth.
 - Warm-up: a dummy Silu preloads the act table (else the first activation
   stalls ~2.5 us on ACT_TABLE_LOAD); 10 dummy matmuls ramp the PE p-state
   during the DMA wait.
 - Phase 1 uses j-outer sweeps after the first f-group (a sweep needs only 2
   free PSUM banks, not 4, killing 2-3 us stalls at f-group boundaries); the
   cold-start f-group is j-inner to match the DMA delivery pace.
"""

import os
import numpy as np
import ml_dtypes

T, D, DFF, E, TOPK = 1024, 2048, 1024, 16, 6
NCORES, P = 8, 128
EPC = E // NCORES  # experts per core

# Set by kernel() after each run: BassKernelResults (exec_time_ns when traced).
LAST_RESULT = None

_PROGRAM_CACHE = {}


def _plan(NT):
    """Phase-1 token slices and f-group width (PSUM bank = 512 f32)."""
    TG = NT * P
    if TG <= 512:
        slices = [(0, TG)]
        FG = 2
    else:
        h = (NT + 1) // 2 * P
        slices = [(0, h), (h, TG - h)]
        FG = 1
    return slices, FG


def _build_program(NT, d=D, dff=DFF):
    import concourse.bacc as bacc
    import concourse.mybir as mybir
    import concourse.tile as tile

    f32 = mybir.dt.float32
    bf16 = mybir.dt.bfloat16
    Silu = mybir.ActivationFunctionType.Silu

    TG = NT * P
    KD = d // P        # k-tiles over D (contraction of W0/W1 matmuls)
    KF = dff // P      # k-tiles over DFF (contraction of W2 matmul)
    DSW = min(512, d)  # output D slice width
    NDS = d // DSW     # output D slices
    slices, FG = _plan(NT)
    FGP = FG * P
    NFG = KF // FG

    nc = bacc.Bacc("TRN2", target_bir_lowering=False, debug=False)

    xt_d = nc.dram_tensor("xt", [EPC, KD, P, TG], bf16,
                          kind="ExternalInput").ap()
    w01_d = nc.dram_tensor("w01", [EPC, NFG, KD, P, 2, FGP], bf16,
                           kind="ExternalInput").ap()
    w2t_d = nc.dram_tensor("w2t", [EPC, NDS, P, KF, DSW], bf16,
                           kind="ExternalInput").ap()
    s0_d = nc.dram_tensor("s0v", [EPC, P, 1], f32, kind="ExternalInput").ap()
    coef_d = nc.dram_tensor("coef", [EPC, NT, P, 1], f32,
                            kind="ExternalInput").ap()
    y_d = nc.dram_tensor("y", [EPC, TG, d], bf16, kind="ExternalOutput").ap()

    big_bufs = 2 if NT <= 4 else 1

    with tile.TileContext(nc) as tc:
        # Alternate bulk DMAs across SP (HWDGE) and gpsimd (SWDGE). Neither
        # sequencer runs compute here, so a launch that blocks on a full
        # descriptor queue can't head-of-line-stall the Act/DVE eltwise
        # (which phase-2 start latency depends on).
        rings = [nc.sync, nc.gpsimd]
        ring_state = [0]

        def ring():
            ring_state[0] ^= 1
            return rings[ring_state[0]]

        with (
            tc.tile_pool(name="xt", bufs=big_bufs) as xt_pool,
            tc.tile_pool(name="w01", bufs=24) as w01_pool,
            tc.tile_pool(name="w2", bufs=3) as w2_pool,
            tc.tile_pool(name="ht", bufs=big_bufs) as ht_pool,
            tc.tile_pool(name="act", bufs=6) as act_pool,
            tc.tile_pool(name="out", bufs=8) as out_pool,
            tc.tile_pool(name="sc", bufs=2) as sc_pool,
            tc.tile_pool(name="pgu", bufs=6, space="PSUM") as pgu_pool,
            tc.tile_pool(name="py", bufs=2, space="PSUM") as py_pool,
        ):
            # warm-up: force the Silu act-table load during the DMA prologue
            # instead of stalling the first real activation (~2.5 us).
            warm_in = sc_pool.tile([P, 1], f32, tag="warm_in")
            warm_out = sc_pool.tile([P, 1], f32, tag="warm_out")
            nc.gpsimd.memset(warm_in[:], 0.0)
            nc.scalar.activation(warm_out[:], warm_in[:], Silu)
            # ... and ramp the PE p-state with dummy matmuls (the PE clock
            # needs ~3us of continuous work to reach 2.4GHz; these run and
            # finish inside the DMA wait, so the real stream starts hot)
            warm_w = sc_pool.tile([P, P], bf16, tag="warm_w")
            warm_x = sc_pool.tile([P, 384], bf16, tag="warm_x")
            nc.gpsimd.memset(warm_w[:], 0.0)
            nc.gpsimd.memset(warm_x[:], 0.0)
            psW = py_pool.tile([P, 512], f32, tag="py", name="psW_warm")
            for wi in range(10):
                nc.tensor.matmul(psW[:, :384], warm_w[:], warm_x[:],
                                 start=True, stop=True)

            for e in range(EPC):
                # --- inputs for this expert ---
                xt = xt_pool.tile([P, KD, TG], bf16, tag="xt")
                s0_sb = sc_pool.tile([P, 1], f32, tag="s0")
                ctiles = sc_pool.tile([P, NT], f32, tag="ctiles")

                # --- phase 1: HT = silu(s0 * W0 xT) * (W1 xT), [DFF, TG] ---
                # j (f-tile within group) is the OUTER loop: a j-sweep only
                # needs its own psG/psU pair, so a new f-group can start as
                # soon as 2 of the previous group's 4 PSUM banks drain
                # (j-inner needed all 4 at k=0 -> 2-3 us stall per boundary).
                ht = ht_pool.tile([P, KF, TG], bf16, tag="ht")
                for fg in range(NFG):
                    psG = [None] * FG
                    psU = [None] * FG
                    w01bs = [None] * KD

                    def alloc_groups(j, e=e, fg=fg, psG=psG, psU=psU):
                        psG[j] = [pgu_pool.tile([P, 512], f32, tag="pgu",
                                                name=f"psG_{e}_{fg}_{j}_{si}")
                                  for si in range(len(slices))]
                        psU[j] = [pgu_pool.tile([P, 512], f32, tag="pgu",
                                                name=f"psU_{e}_{fg}_{j}_{si}")
                                  for si in range(len(slices))]

                    def load_k(k, e=e, fg=fg, w01bs=w01bs):
                        if fg == 0:
                            # interleave x loads with the first f-group's
                            # weight loads: matmul k starts once its own
                            # 96KB xt slice + 128KB w01 block land, not
                            # behind MBs of prefetch
                            ring().dma_start(xt[:, k, :], xt_d[e, k])
                        w01bs[k] = w01_pool.tile(
                            [P, 2, FGP], bf16, tag="w01b",
                            name=f"w01b_{e}_{fg}_{k}")
                        ring().dma_start(w01bs[k][:], w01_d[e, fg, k])

                    def mms(j, k, psG=psG, psU=psU, w01bs=w01bs):
                        for si, (t0, W) in enumerate(slices):
                            nc.tensor.matmul(
                                psG[j][si][:, :W],
                                w01bs[k][:, 0, j * P:(j + 1) * P],
                                xt[:, k, t0:t0 + W],
                                start=(k == 0), stop=(k == KD - 1))
                            nc.tensor.matmul(
                                psU[j][si][:, :W],
                                w01bs[k][:, 1, j * P:(j + 1) * P],
                                xt[:, k, t0:t0 + W],
                                start=(k == 0), stop=(k == KD - 1))

                    def eltwise(j, fg=fg, psG=psG, psU=psU, ht=ht):
                        f = fg * FG + j
                        for si, (t0, W) in enumerate(slices):
                            sig = act_pool.tile([P, 512], f32, tag="sig")
                            nc.scalar.activation(
                                sig[:, :W], psG[j][si][:, :W], Silu,
                                scale=s0_sb[:])
                            nc.vector.tensor_mul(
                                ht[:, f, t0:t0 + W], sig[:, :W],
                                psU[j][si][:, :W])

                    if fg == 0:
                        # cold start: k-outer (j-inner) matches the DMA
                        # delivery pace (4 matmuls per 224KB k-slice), and
                        # all PSUM groups are free at expert start anyway
                        for j in range(FG):
                            alloc_groups(j)
                        for k in range(KD):
                            load_k(k)
                            for j in range(FG):
                                mms(j, k)
                        # small scalars after the first weight-group's
                        # loads: not needed until the first activation
                        nc.sync.dma_start(s0_sb[:], s0_d[e])
                        for m in range(NT):
                            nc.gpsimd.dma_start(ctiles[:, m:m + 1],
                                                coef_d[e, m])
                        for j in range(FG):
                            eltwise(j)
                    else:
                        # steady state: j-outer, so a new f-group only needs
                        # 2 free PSUM banks (not 4) to start its first sweep
                        for j in range(FG):
                            alloc_groups(j)
                            for k in range(KD):
                                if j == 0:
                                    load_k(k)
                                mms(j, k)
                            eltwise(j)

                # --- phase 2: Y = HT^T @ W2^T, scaled per token ---
                for dsi in range(NDS):
                    w2b = w2_pool.tile([P, KF, DSW], bf16, tag="w2b")
                    # On the rings (not SWDGE): ring-queue FIFO order means
                    # these 1MB blocks transfer AFTER the phase-1 weight
                    # stream instead of stealing prologue bandwidth at t=0.
                    ring().dma_start(w2b[:], w2t_d[e, dsi])
                    for m in range(NT):
                        psY = py_pool.tile([P, 512], f32, tag="py",
                                           name=f"psY_{e}_{dsi}_{m}")
                        for k in range(KF):
                            nc.tensor.matmul(
                                psY[:, :DSW], ht[:, k, m * P:(m + 1) * P],
                                w2b[:, k, :],
                                start=(k == 0), stop=(k == KF - 1))
                        ysb = out_pool.tile([P, DSW], bf16, tag="ysb")
                        nc.vector.tensor_scalar_mul(
                            ysb[:], psY[:, :DSW], ctiles[:, m:m + 1])
                        # y always via HWDGE/sync: a SWDGE dma in flight at
                        # program end costs ~10us of gpsimd drain
                        nc.sync.dma_start(
                            y_d[e, m * P:(m + 1) * P,
                                dsi * DSW:(dsi + 1) * DSW], ysb[:])

    nc.compile()
    return nc


def _prep_host(inputs):
    """Host-side dispatch: routing weights, per-expert token gather, layouts."""
    x = np.asarray(inputs["x"], dtype=np.float32)
    w0 = np.asarray(inputs["w0"], dtype=np.float32)
    w1 = np.asarray(inputs["w1"], dtype=np.float32)
    w2 = np.asarray(inputs["w2"], dtype=np.float32)
    s0 = np.asarray(inputs["s0"], dtype=np.float32)
    s1 = np.asarray(inputs["s1"], dtype=np.float32)
    s2 = np.asarray(inputs["s2"], dtype=np.float32)
    se = np.asarray(inputs["selected_experts"]).astype(np.int64)
    rw = np.asarray(inputs["routing_weights"], dtype=np.float32)

    Tn, Dn = x.shape
    En, DFFn, _ = w0.shape
    KD = Dn // P
    KF = DFFn // P
    DSW = min(512, Dn)
    NDS = Dn // DSW

    # combine weight per (expert, token): sum of routing weights over top-k
    cw = np.zeros((En, Tn), np.float32)
    cols = np.arange(Tn)
    for k in range(se.shape[1]):
        np.add.at(cw, (se[:, k], cols), rw[:, k])

    idx = [np.flatnonzero(cw[e] != 0.0) for e in range(En)]
    maxn = max(len(i) for i in idx)
    NT = max(2, -(-maxn // P))  # >=256 padded tokens
    TG = NT * P
    slices, FG = _plan(NT)
    FGP = FG * P
    NFG = KF // FG

    bf = ml_dtypes.bfloat16
    xT = np.ascontiguousarray(x.T)  # [D, T]
    in_maps = []
    for c in range(NCORES):
        xt = np.zeros((EPC, KD, P, TG), bf)
        coef = np.zeros((EPC, TG), np.float32)
        s0v = np.zeros((EPC, P, 1), np.float32)
        w01 = np.empty((EPC, NFG, KD, P, 2, FGP), bf)
        w2t = np.empty((EPC, NDS, P, KF, DSW), bf)
        for j in range(EPC):
            e = c * EPC + j
            ids = idx[e]
            xt[j, :, :, :len(ids)] = xT[:, ids].reshape(KD, P, len(ids))
            coef[j, :len(ids)] = s1[e] * s2[e] * cw[e, ids]
            s0v[j, :, 0] = s0[e]
            # [D, DFF] -> [NFG, KD, P, FGP] blocks, w0/w1 interleaved
            a = w0[e].T.reshape(KD, P, NFG, FGP).transpose(2, 0, 1, 3)
            b = w1[e].T.reshape(KD, P, NFG, FGP).transpose(2, 0, 1, 3)
            w01[j] = np.stack([a, b], axis=3)
            # w2 [D, DFF] -> per (dsi): [P, KF, DSW] partition-major
            # w2t_dev[dsi, p, k, c] = w2T[k*P + p, dsi*DSW + c]
            w2t[j] = w2[e].T.reshape(KF, P, NDS, DSW).transpose(2, 1, 0, 3)
        in_maps.append({
            "xt": xt,
            "w01": w01,
            "w2t": w2t,
            "s0v": s0v,
            "coef": np.ascontiguousarray(coef.reshape(EPC, NT, P, 1)),
        })
    return in_maps, idx, NT, (Tn, Dn, DFFn)


def _combine(results, idx, shapes):
    """Unshard: scatter-add per-expert outputs into the dense [T, D] output."""
    Tn, Dn, _ = shapes
    out = np.zeros((Tn, Dn), np.float32)
    for c in range(NCORES):
        y = results[c]["y"]
        for j in range(EPC):
            e = c * EPC + j
            ids = idx[e]
            if len(ids):
                out[ids] += y[j, :len(ids), :].astype(np.float32)
    return out


def _ensure_axon_ntff_hook():
    """Provide antenv.axon_hooks if the image's antenv stub lacks it."""
    import sys
    import types
    try:
        import antenv.axon_hooks  # noqa: F401
        return
    except ImportError:
        pass
    try:
        import antenv

        mod = types.ModuleType("antenv.axon_hooks")
        _state = {"hook": None}
        mod.set_axon_ntff_profile_hook = lambda h: _state.__setitem__("hook", h)
        mod.get_axon_ntff_profile_hook = lambda: _state["hook"]
        sys.modules["antenv.axon_hooks"] = mod
        antenv.axon_hooks = mod
        try:
            from trn_agent_boot.trn_boot import _ntff_profile_via_ctypes

            so = "/opt/axon/libaxon_pjrt.so"
            if os.path.exists(so):
                mod.set_axon_ntff_profile_hook(_ntff_profile_via_ctypes(so))
        except Exception:
            pass
    except Exception:
        pass


def kernel(**inputs) -> np.ndarray:
    global LAST_RESULT
    _ensure_axon_ntff_hook()
    from concourse.bass_utils import run_bass_kernel_spmd

    in_maps, idx, NT, shapes = _prep_host(inputs)

    key = (NT,) + shapes
    nc = _PROGRAM_CACHE.get(key)
    if nc is None:
        nc = _build_program(NT, d=shapes[1], dff=shapes[2])
        _PROGRAM_CACHE[key] = nc

    res = run_bass_kernel_spmd(nc, in_maps, core_ids=list(range(NCORES)))
    LAST_RESULT = res
    return _combine(res.results, idx, shapes)

